# revision 1
# baseline (speedup 1.0000x reference)
"""Trainium2 Bass kernel for GaussianKernelGCNLayer.

Reference computation (per instance b of 2048 = 8*256):
  wf[b,k,d] = sum_n w[b,n,k] * f[b,n,d]         (n=32 neighbors, k=8 kernels)
  out[b,k,o] = sum_d wf[b,k,d] * CW[k,d,o]      (d=4096, o=512)

Sharding: data-parallel over the 2048 instances -> 256 per core on 8 cores.

Per-core device algorithm (all matmul inputs bf16, fp32 PSUM accumulate):
  Phase 1: for each group g of 4 instances, stack their (32-neighbor)
    features into a [128, 4096] SBUF tile (contract dim = 4*32 = 128
    partitions) and matmul against a host-prebuilt block-diagonal
    weight tile [128, 32] whose 4 diagonal blocks are the per-instance
    [32, 8] neighbour weights.  lhsT = feature d-chunk [128, 128],
    rhs = block-diag weights -> psum [128(d), 32(bi,k)]: this yields
    wf TRANSPOSED (d on partitions) which is exactly the layout phase 2
    needs, with no on-chip transpose.
  Phase 2: for each kernel k: out[b, k*512:+512] = wf_k @ CW_k as 32
    accumulating matmuls over d-chunks; lhsT = wfT[:, c, k, mtile]
    ([128 d, 128 b] contiguous), rhs = CW[k, chunk] ([128 d, 512 o]).
"""

import os
import sys

import numpy as np

try:
    import ml_dtypes
except ImportError:  # pragma: no cover
    ml_dtypes = None

for _p in ("/opt/trn_rl_repo",):
    if _p not in sys.path:
        sys.path.insert(0, _p)

NB, NI, NN, DIN = 8, 256, 32, 4096
NK, DKO = 8, 512
NCORES = 8
BL = NB * NI // NCORES  # 256 instances per core
NGRP = BL // 4          # 64 groups of 4 instances
NCH = DIN // 128        # 32 d-chunks
BF16 = ml_dtypes.bfloat16 if ml_dtypes is not None else None

_cached_nc = None


def _build(repeat=1, phases=(1, 2)):
    from contextlib import ExitStack

    import concourse.bass as bass  # noqa: F401
    import concourse.tile as tile
    from concourse import bacc, mybir

    nc = bacc.Bacc(
        "TRN2",
        target_bir_lowering=False,
        debug=False,
        num_devices=NCORES,
    )

    f_d = nc.dram_tensor(
        "fstack", [NGRP, 128, DIN], mybir.dt.bfloat16, kind="ExternalInput"
    ).ap()
    w_d = nc.dram_tensor(
        "wblk", [NGRP, 128, 32], mybir.dt.bfloat16, kind="ExternalInput"
    ).ap()
    cw_d = nc.dram_tensor(
        "cw", [NK, DIN, DKO], mybir.dt.bfloat16, kind="ExternalInput"
    ).ap()
    out_d = nc.dram_tensor(
        "out", [BL, NK * DKO], mybir.dt.float32, kind="ExternalOutput"
    ).ap()

    with ExitStack() as ctx:
        tc = ctx.enter_context(tile.TileContext(nc))
        const_pool = ctx.enter_context(tc.tile_pool(name="const", bufs=1))
        fpool = ctx.enter_context(tc.tile_pool(name="fpool", bufs=3))
        wpool = ctx.enter_context(tc.tile_pool(name="wpool", bufs=3))
        ps1 = ctx.enter_context(tc.tile_pool(name="ps1", bufs=3, space="PSUM"))
        ps2 = ctx.enter_context(tc.tile_pool(name="ps2", bufs=4, space="PSUM"))
        wtpool = ctx.enter_context(tc.tile_pool(name="wtpool", bufs=8))
        opool = ctx.enter_context(tc.tile_pool(name="opool", bufs=4))

        # Persistent transposed wf: [128 (d%128), chunk, k, g, bi] bf16.
        # For phase 2, wfT[:, c, k, mt*32:(mt+1)*32, :] is a contiguous
        # [128, 128] block -> FWL-eligible weight loads.
        wfT = const_pool.tile(
            [128, NCH, NK, NGRP, 4], mybir.dt.bfloat16, name="wfT"
        )

        if repeat > 1:
            ctx.enter_context(tc.For_i(0, repeat, 1))

        # ---- Phase 1: wfT[d, (bi,k)] per instance-group ----
        for g in range(NGRP):
            fs = fpool.tile([128, DIN], mybir.dt.bfloat16, name="fs")
            nc.sync.dma_start(fs[:], f_d[g, :, :])
            wb = wpool.tile([128, 4, 8], mybir.dt.bfloat16, name="wb")
            nc.sync.dma_start(wb[:], w_d[g, :, :].rearrange("p (bi k) -> p bi k", k=NK))
            for h in range(2):
                pt = ps1.tile([128, 16, 4, 8], mybir.dt.float32, name="pt")
                for cc in range(16):
                    c = h * 16 + cc
                    nc.tensor.matmul(
                        pt[:, cc, :, :],
                        fs[:, c * 128 : (c + 1) * 128],
                        wb[:],
                        start=True,
                        stop=True,
                    )
                # psum [128, 16, bi, k] -> wfT[:, h*16:(h+1)*16, k, g, bi]
                nc.vector.tensor_copy(
                    wfT[:, h * 16 : (h + 1) * 16, :, g, :],
                    pt[:].rearrange("p cc bi k -> p cc k bi"),
                )

        # ---- Phase 2: out = wf @ CW, k-outer, both m-tiles per W pass ----
        for k in range(NK):
            po0 = ps2.tile([128, DKO], mybir.dt.float32, name="po0", tag="po")
            po1 = ps2.tile([128, DKO], mybir.dt.float32, name="po1", tag="po")
            pos = (po0, po1)
            for c in range(NCH):
                wt = wtpool.tile([128, DKO], mybir.dt.bfloat16, name="wt")
                nc.sync.dma_start(wt[:], cw_d[k, c * 128 : (c + 1) * 128, :])
                for mt in range(2):
                    lhs = wfT[:, c, k, mt * 32 : (mt + 1) * 32, :]
                    nc.tensor.matmul(
                        pos[mt][:],
                        lhs,
                        wt[:],
                        start=(c == 0),
                        stop=(c == NCH - 1),
                    )
            for mt in range(2):
                ot = opool.tile([128, DKO], mybir.dt.float32, name="ot")
                nc.vector.tensor_copy(ot[:], pos[mt][:])
                nc.sync.dma_start(
                    out_d[mt * 128 : (mt + 1) * 128, k * DKO : (k + 1) * DKO],
                    ot[:],
                )

    nc.compile()
    return nc


def _prep_inputs(neighbourhood_features, neighbourhood_weights, conv_weight):
    f = np.asarray(neighbourhood_features, dtype=np.float32).reshape(
        NB * NI, NN, DIN
    )
    w = np.asarray(neighbourhood_weights, dtype=np.float32).reshape(NB * NI, NN, NK)
    cw16 = np.ascontiguousarray(np.asarray(conv_weight, dtype=np.float32)).astype(
        BF16
    )
    in_maps = []
    for i in range(NCORES):
        fl = (
            f[i * BL : (i + 1) * BL]
            .reshape(NGRP, 4 * NN, DIN)
            .astype(BF16)
        )
        wl = w[i * BL : (i + 1) * BL].reshape(NGRP, 4, NN, NK)
        wblk = np.zeros((NGRP, 128, 32), dtype=np.float32)
        for bi in range(4):
            wblk[:, bi * 32 : (bi + 1) * 32, bi * 8 : (bi + 1) * 8] = wl[:, bi]
        in_maps.append(
            {
                "fstack": np.ascontiguousarray(fl),
                "wblk": wblk.astype(BF16),
                "cw": cw16,
            }
        )
    return in_maps


def _execute(neighbourhood_features, neighbourhood_weights, conv_weight, trace=False):
    global _cached_nc
    if _cached_nc is None:
        _cached_nc = _build()
    nc = _cached_nc
    from concourse import bass_utils

    in_maps = _prep_inputs(
        neighbourhood_features, neighbourhood_weights, conv_weight
    )
    res = bass_utils.run_bass_kernel_spmd(
        nc, in_maps, core_ids=list(range(NCORES)), trace=trace
    )
    outs = [np.asarray(res.results[i]["out"], dtype=np.float32) for i in range(NCORES)]
    full = np.concatenate(outs, axis=0)
    return full.reshape(NB, NI, NK * DKO), res


def kernel(neighbourhood_features, neighbourhood_weights, conv_weight):
    out, _ = _execute(
        neighbourhood_features, neighbourhood_weights, conv_weight, trace=False
    )
    return out



# revision 4
# speedup vs baseline: 1.1063x; 1.1063x over previous
"""Trainium2 Bass kernel for GaussianKernelGCNLayer.

Reference computation (per instance b of 2048 = 8*256):
  wf[b,k,d] = sum_n w[b,n,k] * f[b,n,d]         (n=32 neighbors, k=8 kernels)
  out[b,k,o] = sum_d wf[b,k,d] * CW[k,d,o]      (d=4096, o=512)

Sharding: data-parallel over the 2048 instances -> 256 per core on 8 cores.

Per-core device algorithm (all matmul inputs bf16, fp32 PSUM accumulate):
  Phase 1: for each group g of 4 instances, stack their (32-neighbor)
    features into a [128, 4096] SBUF tile (contract dim = 4*32 = 128
    partitions) and matmul against a host-prebuilt block-diagonal
    weight tile [128, 32] (k-major columns: col = k*4+bi) -> psum
    [128(d), 32(k,bi)] per d-chunk: wf TRANSPOSED (d on partitions),
    exactly the layout phase 2 needs.  PSUM->SBUF copies are contiguous
    in source and alternate between Vector and Scalar engines.
  Phase 2: for each kernel k: out[b, k*512:+512] = wf_k @ CW_k as 32
    accumulating matmuls; lhsT = wfT[:, c, k, mtile] ([128 d, 128 b]
    contiguous -> FWL), rhs = CW[k, chunk] ([128 d, 512 o]) from
    host-relaid-out 1 MiB contiguous DMA tiles.  Output stored bf16
    (host casts back to fp32).

DMA per iteration per core: 64x1MiB fs + 1x512KiB wblk + 32x1MiB CW
+ 16x128KiB out  (~103 MiB, all transfers >= 512 KiB except out).
"""

import os
import sys

import numpy as np

try:
    import ml_dtypes
except ImportError:  # pragma: no cover
    ml_dtypes = None

for _p in ("/opt/trn_rl_repo",):
    if _p not in sys.path:
        sys.path.insert(0, _p)

NB, NI, NN, DIN = 8, 256, 32, 4096
NK, DKO = 8, 512
NCORES = 8
BL = NB * NI // NCORES  # 256 instances per core
NGRP = BL // 4          # 64 groups of 4 instances
NCH = DIN // 128        # 32 d-chunks
NT = 4                  # CW DMA tiles per kernel-k (8 chunks = 1 MiB each)
BF16 = ml_dtypes.bfloat16 if ml_dtypes is not None else None

_cached_nc = None


def _build(repeat=1, phases=("p1", "p2")):
    from contextlib import ExitStack

    import concourse.bass as bass  # noqa: F401
    import concourse.tile as tile
    from concourse import bacc, mybir

    nc = bacc.Bacc(
        "TRN2",
        target_bir_lowering=False,
        debug=False,
        num_devices=NCORES,
    )

    f_d = nc.dram_tensor(
        "fstack", [NGRP, 128, DIN], mybir.dt.bfloat16, kind="ExternalInput"
    ).ap()
    w_d = nc.dram_tensor(
        "wblk", [128, NGRP, 32], mybir.dt.bfloat16, kind="ExternalInput"
    ).ap()
    # host-relaid CW: [k, t, p, cc, o] with d = t*1024 + cc*128 + p
    cw_d = nc.dram_tensor(
        "cw", [NK, NT, 128, 8, DKO], mybir.dt.bfloat16, kind="ExternalInput"
    ).ap()
    out_d = nc.dram_tensor(
        "out", [BL, NK * DKO], mybir.dt.bfloat16, kind="ExternalOutput"
    ).ap()

    with ExitStack() as ctx:
        tc = ctx.enter_context(tile.TileContext(nc))
        const_pool = ctx.enter_context(tc.tile_pool(name="const", bufs=1))
        fpool = ctx.enter_context(tc.tile_pool(name="fpool", bufs=3))
        ps1 = ctx.enter_context(tc.tile_pool(name="ps1", bufs=4, space="PSUM"))
        ps2 = ctx.enter_context(tc.tile_pool(name="ps2", bufs=4, space="PSUM"))
        wtpool = ctx.enter_context(tc.tile_pool(name="wtpool", bufs=3))
        opool = ctx.enter_context(tc.tile_pool(name="opool", bufs=4))

        # Persistent transposed wf: [128 (d%128), chunk, k, g, bi] bf16.
        # For phase 2, wfT[:, c, k, mt*32:(mt+1)*32, :] is a contiguous
        # [128, 128] block -> FWL-eligible weight loads.
        wfT = const_pool.tile(
            [128, NCH, NK, NGRP, 4], mybir.dt.bfloat16, name="wfT"
        )
        # all 64 groups' block-diag weights, one 512 KiB DMA
        wball = const_pool.tile([128, NGRP, 32], mybir.dt.bfloat16, name="wball")

        if repeat > 1:
            ctx.enter_context(tc.For_i(0, repeat, 1))

        do_p1 = "p1" in phases
        do_p2 = "p2" in phases
        do_dma_only = "dma" in phases

        nc.sync.dma_start(wball[:], w_d[:, :, :])

        # ---- Phase 1: wfT[d, (k,bi)] per instance-group ----
        if do_p1 or do_dma_only:
            for g in range(NGRP):
                fs = fpool.tile([128, DIN], mybir.dt.bfloat16, name="fs")
                nc.sync.dma_start(fs[:], f_d[g, :, :])
                if do_dma_only:
                    continue
                for h in range(2):
                    pt = ps1.tile([128, 16, NK, 4], mybir.dt.float32, name="pt")
                    for cc in range(16):
                        c = h * 16 + cc
                        nc.tensor.matmul(
                            pt[:, cc, :, :],
                            fs[:, c * 128 : (c + 1) * 128],
                            wball[:, g, :],
                            start=True,
                            stop=True,
                        )
                    # psum [128, cc, k, bi] (contiguous src) ->
                    # wfT[:, h*16:(h+1)*16, :, g, :]
                    dst = wfT[:, h * 16 : (h + 1) * 16, :, g, :]
                    if (2 * g + h) % 3:
                        nc.vector.tensor_copy(dst, pt[:])
                    else:
                        nc.scalar.activation(
                            dst, pt[:], mybir.ActivationFunctionType.Copy
                        )

        # ---- Phase 2: out = wf @ CW, k-outer, both m-tiles per W pass ----
        if do_p2 or do_dma_only:
            for k in range(NK):
                po0 = ps2.tile([128, DKO], mybir.dt.float32, name="po0", tag="po")
                po1 = ps2.tile([128, DKO], mybir.dt.float32, name="po1", tag="po")
                pos = (po0, po1)
                for t in range(NT):
                    wt = wtpool.tile([128, 8, DKO], mybir.dt.bfloat16, name="wt")
                    nc.sync.dma_start(wt[:], cw_d[k, t, :, :, :])
                    if do_dma_only:
                        continue
                    for cc in range(8):
                        c = t * 8 + cc
                        for mt in range(2):
                            lhs = wfT[:, c, k, mt * 32 : (mt + 1) * 32, :]
                            nc.tensor.matmul(
                                pos[mt][:],
                                lhs,
                                wt[:, cc, :],
                                start=(c == 0),
                                stop=(c == NCH - 1),
                            )
                for mt in range(2):
                    ot = opool.tile([128, DKO], mybir.dt.bfloat16, name="ot")
                    if not do_dma_only:
                        if mt:
                            nc.vector.tensor_copy(ot[:], pos[mt][:])
                        else:
                            nc.scalar.activation(
                                ot[:], pos[mt][:],
                                mybir.ActivationFunctionType.Copy,
                            )
                    nc.sync.dma_start(
                        out_d[mt * 128 : (mt + 1) * 128, k * DKO : (k + 1) * DKO],
                        ot[:],
                    )

    nc.compile()
    return nc


def _prep_inputs(neighbourhood_features, neighbourhood_weights, conv_weight):
    f = np.asarray(neighbourhood_features, dtype=np.float32).reshape(
        NB * NI, NN, DIN
    )
    w = np.asarray(neighbourhood_weights, dtype=np.float32).reshape(NB * NI, NN, NK)
    cw = np.ascontiguousarray(np.asarray(conv_weight, dtype=np.float32))
    # [k, d, o] -> [k, t, p, cc, o] with d = t*1024 + cc*128 + p
    cwh = np.ascontiguousarray(
        cw.reshape(NK, NT, 8, 128, DKO).transpose(0, 1, 3, 2, 4)
    ).astype(BF16)
    in_maps = []
    for i in range(NCORES):
        fl = (
            f[i * BL : (i + 1) * BL]
            .reshape(NGRP, 4 * NN, DIN)
            .astype(BF16)
        )
        wl = w[i * BL : (i + 1) * BL].reshape(NGRP, 4, NN, NK)
        # block-diag, k-major columns: wblk[p=bi*32+n, g, col=k*4+bi]
        wblk = np.zeros((128, NGRP, 32), dtype=np.float32)
        for bi in range(4):
            for k in range(NK):
                wblk[bi * 32 : (bi + 1) * 32, :, k * 4 + bi] = wl[
                    :, bi, :, k
                ].T
        in_maps.append(
            {
                "fstack": np.ascontiguousarray(fl),
                "wblk": wblk.astype(BF16),
                "cw": cwh,
            }
        )
    return in_maps


def _execute(neighbourhood_features, neighbourhood_weights, conv_weight, trace=False):
    global _cached_nc
    if _cached_nc is None:
        _cached_nc = _build()
    nc = _cached_nc
    from concourse import bass_utils

    in_maps = _prep_inputs(
        neighbourhood_features, neighbourhood_weights, conv_weight
    )
    res = bass_utils.run_bass_kernel_spmd(
        nc, in_maps, core_ids=list(range(NCORES)), trace=trace
    )
    outs = [
        np.asarray(res.results[i]["out"]).astype(np.float32)
        for i in range(NCORES)
    ]
    full = np.concatenate(outs, axis=0)
    return full.reshape(NB, NI, NK * DKO), res


def kernel(neighbourhood_features, neighbourhood_weights, conv_weight):
    out, _ = _execute(
        neighbourhood_features, neighbourhood_weights, conv_weight, trace=False
    )
    return out


# revision 6
# speedup vs baseline: 1.3103x; 1.1844x over previous
"""Trainium2 Bass kernel for GaussianKernelGCNLayer.

Reference computation (per instance b of 2048 = 8*256):
  wf[b,k,d] = sum_n w[b,n,k] * f[b,n,d]         (n=32 neighbors, k=8 kernels)
  out[b,k,o] = sum_d wf[b,k,d] * CW[k,d,o]      (d=4096, o=512)

Sharding: data-parallel over the 2048 instances -> 256 per core on 8 cores.

Per-core device algorithm (all matmul inputs bf16, fp32 PSUM accumulate):
  Phase 1: for each group g of 4 instances, stack their (32-neighbor)
    features into a [128, 4096] SBUF tile (contract dim = 4*32 = 128
    partitions) and matmul against a host-prebuilt block-diagonal
    weight tile [128, 32] (k-major columns: col = k*4+bi) -> psum
    [128(d), 32(k,bi)] per d-chunk: wf TRANSPOSED (d on partitions),
    exactly the layout phase 2 needs.  PSUM->SBUF copies are contiguous
    in source and alternate between Vector and Scalar engines.
  Phase 2: for each kernel k: out[b, k*512:+512] = wf_k @ CW_k as 32
    accumulating matmuls; lhsT = wfT[:, c, k, mtile] ([128 d, 128 b]
    contiguous -> FWL), rhs = CW[k, chunk] ([128 d, 512 o]) from
    host-relaid-out 1 MiB contiguous DMA tiles.  Output stored bf16
    (host casts back to fp32).

DMA per iteration per core: 64x1MiB fs + 1x512KiB wblk + 32x1MiB CW
+ 16x128KiB out  (~103 MiB, all transfers >= 512 KiB except out).
"""

import os
import sys

import numpy as np

try:
    import ml_dtypes
except ImportError:  # pragma: no cover
    ml_dtypes = None

for _p in ("/opt/trn_rl_repo",):
    if _p not in sys.path:
        sys.path.insert(0, _p)

NB, NI, NN, DIN = 8, 256, 32, 4096
NK, DKO = 8, 512
NCORES = 8
BL = NB * NI // NCORES  # 256 instances per core
NGRP = BL // 4          # 64 groups of 4 instances
NCH = DIN // 128        # 32 d-chunks
NT = 4                  # CW DMA tiles per kernel-k (8 chunks = 1 MiB each)
BF16 = ml_dtypes.bfloat16 if ml_dtypes is not None else None

_cached_nc = None


def _build(repeat=1, phases=("p1", "p2")):
    from contextlib import ExitStack

    import concourse.bass as bass  # noqa: F401
    import concourse.tile as tile
    from concourse import bacc, mybir

    nc = bacc.Bacc(
        "TRN2",
        target_bir_lowering=False,
        debug=False,
        num_devices=NCORES,
    )

    f_d = nc.dram_tensor(
        "fstack", [NGRP, 128, DIN], mybir.dt.bfloat16, kind="ExternalInput"
    ).ap()
    w_d = nc.dram_tensor(
        "wblk", [128, NGRP, 32], mybir.dt.bfloat16, kind="ExternalInput"
    ).ap()
    # host-relaid CW: [k, t, p, cc, o] with d = t*1024 + cc*128 + p
    cw_d = nc.dram_tensor(
        "cw", [NK, NT, 128, 8, DKO], mybir.dt.bfloat16, kind="ExternalInput"
    ).ap()
    out_d = nc.dram_tensor(
        "out", [BL, NK * DKO], mybir.dt.bfloat16, kind="ExternalOutput"
    ).ap()

    with ExitStack() as ctx:
        tc = ctx.enter_context(tile.TileContext(nc))
        const_pool = ctx.enter_context(tc.tile_pool(name="const", bufs=1))
        fpool = ctx.enter_context(tc.tile_pool(name="fpool", bufs=3))
        ps1 = ctx.enter_context(tc.tile_pool(name="ps1", bufs=4, space="PSUM"))
        ps2 = ctx.enter_context(tc.tile_pool(name="ps2", bufs=4, space="PSUM"))
        wtpool = ctx.enter_context(tc.tile_pool(name="wtpool", bufs=3))
        opool = ctx.enter_context(tc.tile_pool(name="opool", bufs=4))

        # Persistent transposed wf: [128 (d%128), chunk, k, g, bi] bf16.
        # For phase 2, wfT[:, c, k, mt*32:(mt+1)*32, :] is a contiguous
        # [128, 128] block -> FWL-eligible weight loads.
        wfT = const_pool.tile(
            [128, NCH, NK, NGRP, 4], mybir.dt.bfloat16, name="wfT"
        )
        # all 64 groups' block-diag weights, one 512 KiB DMA
        wball = const_pool.tile([128, NGRP, 32], mybir.dt.bfloat16, name="wball")

        if repeat > 1:
            ctx.enter_context(tc.For_i(0, repeat, 1))

        do_p1 = "p1" in phases
        do_p2 = "p2" in phases
        do_dma_only = "dma" in phases

        nc.sync.dma_start(wball[:], w_d[:, :, :])

        # ---- Phase 1: wfT[d, (k,bi)] per instance-group ----
        if do_p1 or do_dma_only:
            for g in range(NGRP):
                fs = fpool.tile([128, DIN], mybir.dt.bfloat16, name="fs")
                nc.sync.dma_start(fs[:], f_d[g, :, :])
                if do_dma_only:
                    continue
                for h in range(2):
                    pt = ps1.tile([128, 16, NK, 4], mybir.dt.float32, name="pt")
                    for cc in range(16):
                        c = h * 16 + cc
                        nc.tensor.matmul(
                            pt[:, cc, :, :],
                            fs[:, c * 128 : (c + 1) * 128],
                            wball[:, g, :],
                            start=True,
                            stop=True,
                        )
                    # psum [128, cc, k, bi] (contiguous src) ->
                    # wfT[:, h*16:(h+1)*16, :, g, :]
                    dst = wfT[:, h * 16 : (h + 1) * 16, :, g, :]
                    if (2 * g + h) % 3:
                        nc.vector.tensor_copy(dst, pt[:])
                    else:
                        nc.scalar.activation(
                            dst, pt[:], mybir.ActivationFunctionType.Copy
                        )

        # ---- Phase 2: out = wf @ CW, k-outer, both m-tiles per W pass ----
        if do_p2 or do_dma_only:
            for k in range(NK):
                if not do_dma_only:
                    po0 = ps2.tile(
                        [128, DKO], mybir.dt.float32, name="po0", tag="po"
                    )
                    po1 = ps2.tile(
                        [128, DKO], mybir.dt.float32, name="po1", tag="po"
                    )
                    pos = (po0, po1)
                for t in range(NT):
                    wt = wtpool.tile([128, 8, DKO], mybir.dt.bfloat16, name="wt")
                    nc.sync.dma_start(wt[:], cw_d[k, t, :, :, :])
                    if do_dma_only:
                        continue
                    for cc in range(8):
                        c = t * 8 + cc
                        for mt in range(2):
                            lhs = wfT[:, c, k, mt * 32 : (mt + 1) * 32, :]
                            nc.tensor.matmul(
                                pos[mt][:],
                                lhs,
                                wt[:, cc, :],
                                start=(c == 0),
                                stop=(c == NCH - 1),
                            )
                if do_dma_only:
                    continue
                for mt in range(2):
                    ot = opool.tile([128, DKO], mybir.dt.bfloat16, name="ot")
                    if mt:
                        nc.vector.tensor_copy(ot[:], pos[mt][:])
                    else:
                        nc.scalar.activation(
                            ot[:], pos[mt][:],
                            mybir.ActivationFunctionType.Copy,
                        )
                    nc.sync.dma_start(
                        out_d[mt * 128 : (mt + 1) * 128, k * DKO : (k + 1) * DKO],
                        ot[:],
                    )

    nc.compile()
    return nc


def _prep_inputs(neighbourhood_features, neighbourhood_weights, conv_weight):
    f = np.asarray(neighbourhood_features, dtype=np.float32).reshape(
        NB * NI, NN, DIN
    )
    w = np.asarray(neighbourhood_weights, dtype=np.float32).reshape(NB * NI, NN, NK)
    cw = np.ascontiguousarray(np.asarray(conv_weight, dtype=np.float32))
    # [k, d, o] -> [k, t, p, cc, o] with d = t*1024 + cc*128 + p
    cwh = np.ascontiguousarray(
        cw.reshape(NK, NT, 8, 128, DKO).transpose(0, 1, 3, 2, 4)
    ).astype(BF16)
    in_maps = []
    for i in range(NCORES):
        fl = (
            f[i * BL : (i + 1) * BL]
            .reshape(NGRP, 4 * NN, DIN)
            .astype(BF16)
        )
        wl = w[i * BL : (i + 1) * BL].reshape(NGRP, 4, NN, NK)
        # block-diag, k-major columns: wblk[p=bi*32+n, g, col=k*4+bi]
        wblk = np.zeros((128, NGRP, 32), dtype=np.float32)
        for bi in range(4):
            for k in range(NK):
                wblk[bi * 32 : (bi + 1) * 32, :, k * 4 + bi] = wl[
                    :, bi, :, k
                ].T
        in_maps.append(
            {
                "fstack": np.ascontiguousarray(fl),
                "wblk": wblk.astype(BF16),
                "cw": cwh,
            }
        )
    return in_maps


def _execute(neighbourhood_features, neighbourhood_weights, conv_weight, trace=False):
    global _cached_nc
    if _cached_nc is None:
        _cached_nc = _build()
    nc = _cached_nc
    from concourse import bass_utils

    in_maps = _prep_inputs(
        neighbourhood_features, neighbourhood_weights, conv_weight
    )
    res = bass_utils.run_bass_kernel_spmd(
        nc, in_maps, core_ids=list(range(NCORES)), trace=trace
    )
    outs = [
        np.asarray(res.results[i]["out"]).astype(np.float32)
        for i in range(NCORES)
    ]
    full = np.concatenate(outs, axis=0)
    return full.reshape(NB, NI, NK * DKO), res


def kernel(neighbourhood_features, neighbourhood_weights, conv_weight):
    out, _ = _execute(
        neighbourhood_features, neighbourhood_weights, conv_weight, trace=False
    )
    return out
